# revision 32
# baseline (speedup 1.0000x reference)
"""AGNNConv on 8 TRN2 NeuronCores — pure-compute streaming design.

This platform (axon/PJRT TRN2) has no usable data-dependent DMA: the
custom SWDGE gather/scatter ucode crashes the device and the generic
indirect DMA path is a ~66us/call software queue.  So the kernel is
built exclusively from streaming DMA + compute engines:

  - Host (pure layout, no numerics): partition edges by dst subwindow of
    64 nodes (196 per core, window-aligned core ranges of 12544 nodes),
    pad each subwindow's edge list to TPW tiles of 128 edge slots, and
    materialize per-edge operand rows fs = feat[src], fd = feat[dst] in
    the exact SBUF layout the device consumes ("node features
    replicated per edge" — the extreme of the sharding hint).
  - Device per batch of subwindows:
      ss_s, ss_d row sums of squares (raw rows -> norms, same math as
      reference), cos = sum(fs*fd) * rsqrt(max(ss_s*ss_d, eps)),
      p = exp(beta*cos)           (softmax max-subtraction dropped:
                                   |beta*cos| <= |beta|, well-conditioned,
                                   mathematically identical)
      payload = [p*fs | p] bf16
      scatter: per 128-edge tile a one-hot matrix A[e, m] =
      (dst%64 == m) built on DVE, and PE matmul A^T @ payload
      accumulates [64 nodes, 33] in PSUM across the subwindow's tiles;
      two subwindows share one PSUM tile (partition halves).
      out = msg / s on evacuation.
  - Pad edges get d8 = 255 -> all-zero one-hot row -> contribute
    nothing.  Zero fs/fd pad rows stay finite through the norm chain.
"""

import sys

if "/opt/trn_rl_repo" not in sys.path:
    sys.path.insert(0, "/opt/trn_rl_repo")

import numpy as np

# Problem constants (hardcoded per harness contract)
N_NODES = 100000
N_EDGES = 1600000
D = 32
NCORES = 8
WSZ = 64           # dst subwindow size (one-hot width)
NW = 196           # subwindows per core
NLOC = NW * WSZ    # 12544 nodes per core (window-aligned; trimmed on host)
TPW = 9            # tiles (128 edge slots) per subwindow; cap 1152 >= max 1133
WB = 14            # subwindows per compute batch (196 = 14*14), even
PW = D + 1         # payload width


def build_graph(nw, tpw, wb, wsz=WSZ, d=D, repeat=1,
                skip_pe=False, skip_a=False, skip_norm=False, skip_cos=False):
    import concourse.bass as bass
    import concourse.tile as tile
    from concourse import bacc, mybir
    from contextlib import nullcontext

    f32 = mybir.dt.float32
    bf16 = mybir.dt.bfloat16
    X = mybir.AxisListType.X
    ADD = mybir.AluOpType.add
    ISEQ = mybir.AluOpType.is_equal

    assert nw % wb == 0 and wb % 2 == 0
    nb = nw // wb
    tb = wb * tpw  # tiles per batch
    npair = nw // 2

    nc = bacc.Bacc(None, target_bir_lowering=False, debug=False)
    fs_p = nc.declare_dram_parameter("fs", [128, nw, tpw, d], bf16, isOutput=False)
    fd_p = nc.declare_dram_parameter("fd", [128, nw, tpw, d], bf16, isOutput=False)
    ah_p = nc.declare_dram_parameter("ah", [128, nw, tpw, wsz], bf16, isOutput=False)
    beta_p = nc.declare_dram_parameter("beta", [1], f32, isOutput=False)
    out_p = nc.declare_dram_parameter("out", [nw * wsz, d], f32, isOutput=True)

    outR = out_p[:].rearrange("(j m) c -> m j c", m=128)

    with tile.TileContext(nc) as tc:
        with tc.tile_pool(name="singles", bufs=1) as singles:
            beta_sb = singles.tile([128, 1], f32)
            nc.sync.dma_start(out=beta_sb[:], in_=beta_p[:].to_broadcast([128, 1]))
            eps_sb = singles.tile([128, 1], f32)
            nc.vector.memset(eps_sb[:], 1e-24)
            obuf = singles.tile([128, npair, d], f32)
            if skip_pe:
                nc.vector.memset(obuf[:], 0.0)

            with (
                tc.tile_pool(name="inp", bufs=2) as inp,
                tc.tile_pool(name="ap_", bufs=2) as ap_,
                tc.tile_pool(name="med", bufs=3) as med,
                tc.tile_pool(name="sml", bufs=4) as sml,
                tc.tile_pool(name="ps_", bufs=4, space="PSUM") as ps_,
                tc.For_i(0, repeat, 1) if repeat > 1 else nullcontext(),
            ):
                for b in range(nb):
                    ws = slice(b * wb, (b + 1) * wb)
                    fs_t = inp.tile([128, wb, tpw, d], bf16)
                    nc.sync.dma_start(out=fs_t[:], in_=fs_p[:, ws, :, :])
                    fd_t = inp.tile([128, wb, tpw, d], bf16)
                    nc.scalar.dma_start(out=fd_t[:], in_=fd_p[:, ws, :, :])
                    # one-hot A[e-partition, m] per tile (host-built, bf16)
                    A_t = ap_.tile([128, wb, tpw, wsz], bf16)
                    if not skip_a:
                        nc.sync.dma_start(out=A_t[:], in_=ah_p[:, ws, :, :])
                    elif b == 0:
                        nc.vector.memset(A_t[:], 0.0)
                    A_f = A_t[:].rearrange("i w t m -> i (w t) m")

                    fsf = fs_t[:].rearrange("i w t c -> i (w t) c")
                    fdf = fd_t[:].rearrange("i w t c -> i (w t) c")

                    # fused products: [fs*fd | fs*fs | fd*fd] -> one reduce
                    cn = sml.tile([128, tb, 1], f32)
                    if not (skip_cos or skip_norm):
                        sq3 = med.tile([128, tb, 3, d], bf16)
                        nc.gpsimd.tensor_mul(sq3[:, :, 0, :], fsf, fdf)
                        nc.gpsimd.tensor_mul(sq3[:, :, 1, :], fsf, fsf)
                        nc.gpsimd.tensor_mul(sq3[:, :, 2, :], fdf, fdf)
                        red = sml.tile([128, tb, 3], f32)
                        nc.vector.tensor_reduce(red[:], sq3[:], axis=X, op=ADD)
                        cos = red[:, :, 0:1]
                        # rn = 1/sqrt(ss_s*ss_d + eps) = exp(-0.5*ln(.))
                        # (ln+exp share one Act table set: no table reloads)
                        ssp = sml.tile([128, tb, 1], f32)
                        nc.vector.tensor_mul(ssp[:], red[:, :, 1:2], red[:, :, 2:3])
                        lg = sml.tile([128, tb, 1], f32)
                        nc.scalar.activation(
                            lg[:], ssp[:], mybir.ActivationFunctionType.Ln,
                            bias=eps_sb[:],
                        )
                        rn = sml.tile([128, tb, 1], f32)
                        nc.scalar.activation(
                            rn[:], lg[:], mybir.ActivationFunctionType.Exp,
                            scale=-0.5,
                        )
                        nc.vector.tensor_mul(cn[:], cos, rn[:])
                    elif b == 0:
                        nc.vector.memset(cn[:], 0.5)

                    # p = exp(beta*cn)
                    p_t = sml.tile([128, tb, 1], bf16)
                    nc.scalar.activation(
                        p_t[:], cn[:], mybir.ActivationFunctionType.Exp,
                        scale=beta_sb[:],
                    )

                    # payload [p*fs | p] in bf16
                    pay = med.tile([128, tb, PW], bf16)
                    nc.gpsimd.tensor_mul(
                        pay[:, :, 0:d], fsf, p_t[:].to_broadcast([128, tb, d])
                    )
                    nc.vector.tensor_copy(out=pay[:, :, d : d + 1], in_=p_t[:])

                    # scatter: PSUM accumulation; 2 subwindows per PSUM tile
                    hb = wb // 2
                    stg = med.tile([128, hb, PW], f32)
                    for pj in range(hb if not skip_pe else 0):
                        ps = ps_.tile([128, PW], f32)
                        for h in range(2):
                            wj = pj * 2 + h
                            for t in range(tpw):
                                ti = wj * tpw + t
                                nc.tensor.matmul(
                                    ps[h * wsz : (h + 1) * wsz, :],
                                    lhsT=A_f[:, ti, :],
                                    rhs=pay[:, ti, :],
                                    start=(t == 0),
                                    stop=(t == tpw - 1),
                                )
                        nc.scalar.copy(out=stg[:, pj, :], in_=ps[:])
                    if not skip_pe:
                        scb = sml.tile([128, hb, 1], f32)
                        nc.vector.tensor_scalar_max(
                            scb[:], stg[:, :, d : d + 1], 1e-30
                        )
                        rcb = sml.tile([128, hb, 1], f32)
                        nc.vector.reciprocal(rcb[:], scb[:])
                        nc.vector.tensor_mul(
                            obuf[:, b * hb : (b + 1) * hb, :],
                            stg[:, :, 0:d],
                            rcb[:].to_broadcast([128, hb, d]),
                        )

            nc.sync.dma_start(out=outR[:, :, :], in_=obuf[:])

    nc.compile()
    return nc


def host_prep(feat, beta, src, dst, ncores, nw, tpw, d, wsz=WSZ):
    """Pure index/layout prep. Returns per-core input maps."""
    import ml_dtypes

    feat = np.ascontiguousarray(np.asarray(feat, dtype=np.float32))
    beta = np.ascontiguousarray(np.asarray(beta, dtype=np.float32))
    src = np.asarray(src).astype(np.int64)
    dst = np.asarray(dst).astype(np.int64)
    cap = tpw * 128

    win = dst // wsz                       # global subwindow id
    order = np.argsort(win, kind="stable")
    src_s, dst_s = src[order], dst[order]
    win_s = win[order]
    wcnt = np.bincount(win_s, minlength=ncores * nw)
    assert wcnt.max() <= cap, f"window overflow: {wcnt.max()} > {cap}"
    starts = np.concatenate([[0], np.cumsum(wcnt)[:-1]])
    rank = np.arange(src_s.size) - starts[win_s]

    feat_bf = feat.astype(ml_dtypes.bfloat16)
    eye = np.eye(wsz, dtype=ml_dtypes.bfloat16)

    in_maps = []
    for c in range(ncores):
        lo_w, hi_w = c * nw, (c + 1) * nw
        sel = (win_s >= lo_w) & (win_s < hi_w)
        e_src, e_dst, e_win, e_rank = (
            src_s[sel], dst_s[sel], win_s[sel] - lo_w, rank[sel],
        )
        t_ = e_rank // 128
        i_ = e_rank % 128

        fs = np.zeros((128, nw, tpw, d), dtype=ml_dtypes.bfloat16)
        fd = np.zeros((128, nw, tpw, d), dtype=ml_dtypes.bfloat16)
        ah = np.zeros((128, nw, tpw, wsz), dtype=ml_dtypes.bfloat16)
        fs[i_, e_win, t_] = feat_bf[e_src]
        fd[i_, e_win, t_] = feat_bf[e_dst]
        ah[i_, e_win, t_] = eye[e_dst % wsz]

        in_maps.append(
            {
                "fs": fs,
                "fd": fd,
                "ah": ah,
                "beta": beta,
            }
        )
    return in_maps


_CACHED = {}


def kernel(feat, beta, src, dst):
    from concourse.bass_utils import run_bass_kernel_spmd

    # adaptive tile capacity: expected inputs need TPW=9, but tolerate
    # denser dst windows by bumping the per-window tile count
    dst_a = np.asarray(dst).astype(np.int64)
    wmax = int(np.bincount(dst_a // WSZ, minlength=NCORES * NW).max())
    tpw = max(TPW, -(-wmax // 128))

    in_maps = host_prep(feat, beta, src, dst, NCORES, NW, tpw, D)
    key = ("nc", tpw)
    if key not in _CACHED:
        _CACHED[key] = build_graph(NW, tpw, WB)
    nc = _CACHED[key]
    res = run_bass_kernel_spmd(nc, in_maps, list(range(NCORES))).results
    full = np.concatenate([res[c]["out"] for c in range(NCORES)], axis=0)
    return full[:N_NODES].astype(np.float32)


# revision 35
# speedup vs baseline: 1.4484x; 1.4484x over previous
"""AGNNConv on 8 TRN2 NeuronCores — pure-compute streaming design.

This platform (axon/PJRT TRN2) has no usable data-dependent DMA: the
custom SWDGE gather/scatter ucode crashes the device and the generic
indirect DMA path is a ~66us/call software queue.  So the kernel is
built exclusively from streaming DMA + compute engines:

  - Host (pure layout, no numerics): partition edges by dst subwindow of
    64 nodes (196 per core, window-aligned core ranges of 12544 nodes),
    pad each subwindow's edge list to TPW tiles of 128 edge slots, and
    materialize per-edge operand rows fs = feat[src], fd = feat[dst] in
    the exact SBUF layout the device consumes ("node features
    replicated per edge" — the extreme of the sharding hint).
  - Device per batch of subwindows:
      ss_s, ss_d row sums of squares (raw rows -> norms, same math as
      reference), cos = sum(fs*fd) * rsqrt(max(ss_s*ss_d, eps)),
      p = exp(beta*cos)           (softmax max-subtraction dropped:
                                   |beta*cos| <= |beta|, well-conditioned,
                                   mathematically identical)
      payload = [p*fs | p] bf16
      scatter: per 128-edge tile a one-hot matrix A[e, m] =
      (dst%64 == m) built on DVE, and PE matmul A^T @ payload
      accumulates [64 nodes, 33] in PSUM across the subwindow's tiles;
      two subwindows share one PSUM tile (partition halves).
      out = msg / s on evacuation.
  - Pad edges get d8 = 255 -> all-zero one-hot row -> contribute
    nothing.  Zero fs/fd pad rows stay finite through the norm chain.
"""

import sys

if "/opt/trn_rl_repo" not in sys.path:
    sys.path.insert(0, "/opt/trn_rl_repo")

import numpy as np

# Problem constants (hardcoded per harness contract)
N_NODES = 100000
N_EDGES = 1600000
D = 32
NCORES = 8
WSZ = 64           # dst subwindow size (one-hot width)
NW = 196           # subwindows per core
NLOC = NW * WSZ    # 12544 nodes per core (window-aligned; trimmed on host)
TPW = 9            # tiles (128 edge slots) per subwindow; cap 1152 >= max 1133
WB = 14            # subwindows per compute batch (196 = 14*14), even
PW = D + 1         # payload width


def build_graph(nw, tpw, wb, wsz=WSZ, d=D, repeat=1,
                skip_pe=False, skip_a=False, skip_norm=False, skip_cos=False):
    import concourse.bass as bass
    import concourse.tile as tile
    from concourse import bacc, mybir
    from contextlib import nullcontext

    f32 = mybir.dt.float32
    bf16 = mybir.dt.bfloat16
    X = mybir.AxisListType.X
    ADD = mybir.AluOpType.add
    ISEQ = mybir.AluOpType.is_equal

    assert nw % wb == 0 and wb % 2 == 0
    nb = nw // wb
    tb = wb * tpw  # tiles per batch
    npair = nw // 2

    nc = bacc.Bacc(None, target_bir_lowering=False, debug=False)
    fs_p = nc.declare_dram_parameter("fs", [128, nw, tpw, d], bf16, isOutput=False)
    fd_p = nc.declare_dram_parameter("fd", [128, nw, tpw, d], bf16, isOutput=False)
    ah_p = nc.declare_dram_parameter("ah", [128, nw, tpw, wsz], bf16, isOutput=False)
    beta_p = nc.declare_dram_parameter("beta", [1], f32, isOutput=False)
    out_p = nc.declare_dram_parameter("out", [nw * wsz, d], f32, isOutput=True)

    outR = out_p[:].rearrange("(j m) c -> m j c", m=128)

    with tile.TileContext(nc) as tc:
        with tc.tile_pool(name="singles", bufs=1) as singles:
            beta_sb = singles.tile([128, 1], f32)
            nc.sync.dma_start(out=beta_sb[:], in_=beta_p[:].to_broadcast([128, 1]))
            eps_sb = singles.tile([128, 1], f32)
            nc.vector.memset(eps_sb[:], 1e-24)
            obuf = singles.tile([128, npair, d], f32)
            if skip_pe:
                nc.vector.memset(obuf[:], 0.0)

            with (
                tc.tile_pool(name="inp", bufs=3) as inp,
                tc.tile_pool(name="ap_", bufs=2) as ap_,
                tc.tile_pool(name="med", bufs=2) as med,
                tc.tile_pool(name="sml", bufs=4) as sml,
                tc.tile_pool(name="ps_", bufs=4, space="PSUM") as ps_,
                tc.For_i(0, repeat, 1) if repeat > 1 else nullcontext(),
            ):
                for b in range(nb):
                    ws = slice(b * wb, (b + 1) * wb)
                    fs_t = inp.tile([128, wb, tpw, d], bf16)
                    nc.sync.dma_start(out=fs_t[:], in_=fs_p[:, ws, :, :])
                    fd_t = inp.tile([128, wb, tpw, d], bf16)
                    nc.scalar.dma_start(out=fd_t[:], in_=fd_p[:, ws, :, :])
                    # one-hot A[e-partition, m] per tile (host-built, bf16)
                    A_t = ap_.tile([128, wb, tpw, wsz], bf16)
                    if not skip_a:
                        nc.sync.dma_start(out=A_t[:], in_=ah_p[:, ws, :, :])
                    elif b == 0:
                        nc.vector.memset(A_t[:], 0.0)
                    A_f = A_t[:].rearrange("i w t m -> i (w t) m")

                    fsf = fs_t[:].rearrange("i w t c -> i (w t) c")
                    fdf = fd_t[:].rearrange("i w t c -> i (w t) c")

                    # cos numerator
                    cos = sml.tile([128, tb, 1], f32)
                    if not skip_cos:
                        prod = med.tile([128, tb, d], bf16)
                        nc.gpsimd.tensor_mul(prod[:], fsf, fdf)
                        nc.vector.tensor_reduce(cos[:], prod[:], axis=X, op=ADD)
                    elif b == 0:
                        nc.vector.memset(cos[:], 0.5)

                    cn = sml.tile([128, tb, 1], f32)
                    if not skip_norm:
                        # squared norms (Pool engine mults, DVE reduces)
                        sq = med.tile([128, tb, d], bf16)
                        nc.gpsimd.tensor_mul(sq[:], fsf, fsf)
                        ss_s = sml.tile([128, tb, 1], f32)
                        nc.vector.tensor_reduce(ss_s[:], sq[:], axis=X, op=ADD)
                        sq2 = med.tile([128, tb, d], bf16)
                        nc.gpsimd.tensor_mul(sq2[:], fdf, fdf)
                        ss_d = sml.tile([128, tb, 1], f32)
                        nc.vector.tensor_reduce(ss_d[:], sq2[:], axis=X, op=ADD)

                        # rn = 1/sqrt(ss_s*ss_d + eps) = exp(-0.5*ln(.))
                        # (ln+exp share one Act table set: no table reloads)
                        ssp = sml.tile([128, tb, 1], f32)
                        nc.vector.tensor_mul(ssp[:], ss_s[:], ss_d[:])
                        lg = sml.tile([128, tb, 1], f32)
                        nc.scalar.activation(
                            lg[:], ssp[:], mybir.ActivationFunctionType.Ln,
                            bias=eps_sb[:],
                        )
                        rn = sml.tile([128, tb, 1], f32)
                        nc.scalar.activation(
                            rn[:], lg[:], mybir.ActivationFunctionType.Exp,
                            scale=-0.5,
                        )
                        nc.vector.tensor_mul(cn[:], cos[:], rn[:])
                    else:
                        nc.vector.tensor_copy(out=cn[:], in_=cos[:])

                    # p = exp(beta*cn)
                    p_t = sml.tile([128, tb, 1], bf16)
                    nc.scalar.activation(
                        p_t[:], cn[:], mybir.ActivationFunctionType.Exp,
                        scale=beta_sb[:],
                    )

                    # payload [p*fs | p] in bf16
                    pay = med.tile([128, tb, PW], bf16)
                    nc.gpsimd.tensor_mul(
                        pay[:, :, 0:d], fsf, p_t[:].to_broadcast([128, tb, d])
                    )
                    nc.vector.tensor_copy(out=pay[:, :, d : d + 1], in_=p_t[:])

                    # scatter: PSUM accumulation; 2 subwindows per PSUM tile
                    hb = wb // 2
                    stg = med.tile([128, hb, PW], f32)
                    for pj in range(hb if not skip_pe else 0):
                        ps = ps_.tile([128, PW], f32)
                        for h in range(2):
                            wj = pj * 2 + h
                            for t in range(tpw):
                                ti = wj * tpw + t
                                nc.tensor.matmul(
                                    ps[h * wsz : (h + 1) * wsz, :],
                                    lhsT=A_f[:, ti, :],
                                    rhs=pay[:, ti, :],
                                    start=(t == 0),
                                    stop=(t == tpw - 1),
                                )
                        nc.scalar.copy(out=stg[:, pj, :], in_=ps[:])
                    if not skip_pe:
                        scb = sml.tile([128, hb, 1], f32)
                        nc.vector.tensor_scalar_max(
                            scb[:], stg[:, :, d : d + 1], 1e-30
                        )
                        rcb = sml.tile([128, hb, 1], f32)
                        nc.vector.reciprocal(rcb[:], scb[:])
                        nc.vector.tensor_mul(
                            obuf[:, b * hb : (b + 1) * hb, :],
                            stg[:, :, 0:d],
                            rcb[:].to_broadcast([128, hb, d]),
                        )

            nc.sync.dma_start(out=outR[:, :, :], in_=obuf[:])

    nc.compile()
    return nc


def host_prep(feat, beta, src, dst, ncores, nw, tpw, d, wsz=WSZ):
    """Pure index/layout prep. Returns per-core input maps."""
    import ml_dtypes

    feat = np.ascontiguousarray(np.asarray(feat, dtype=np.float32))
    beta = np.ascontiguousarray(np.asarray(beta, dtype=np.float32))
    src = np.asarray(src).astype(np.int64)
    dst = np.asarray(dst).astype(np.int64)
    cap = tpw * 128

    win = dst // wsz                       # global subwindow id
    order = np.argsort(win, kind="stable")
    src_s, dst_s = src[order], dst[order]
    win_s = win[order]
    wcnt = np.bincount(win_s, minlength=ncores * nw)
    assert wcnt.max() <= cap, f"window overflow: {wcnt.max()} > {cap}"
    starts = np.concatenate([[0], np.cumsum(wcnt)[:-1]])
    rank = np.arange(src_s.size) - starts[win_s]

    feat_bf = feat.astype(ml_dtypes.bfloat16)
    eye = np.eye(wsz, dtype=ml_dtypes.bfloat16)

    in_maps = []
    for c in range(ncores):
        lo_w, hi_w = c * nw, (c + 1) * nw
        sel = (win_s >= lo_w) & (win_s < hi_w)
        e_src, e_dst, e_win, e_rank = (
            src_s[sel], dst_s[sel], win_s[sel] - lo_w, rank[sel],
        )
        t_ = e_rank // 128
        i_ = e_rank % 128

        fs = np.zeros((128, nw, tpw, d), dtype=ml_dtypes.bfloat16)
        fd = np.zeros((128, nw, tpw, d), dtype=ml_dtypes.bfloat16)
        ah = np.zeros((128, nw, tpw, wsz), dtype=ml_dtypes.bfloat16)
        fs[i_, e_win, t_] = feat_bf[e_src]
        fd[i_, e_win, t_] = feat_bf[e_dst]
        ah[i_, e_win, t_] = eye[e_dst % wsz]

        in_maps.append(
            {
                "fs": fs,
                "fd": fd,
                "ah": ah,
                "beta": beta,
            }
        )
    return in_maps


_CACHED = {}


def kernel(feat, beta, src, dst):
    from concourse.bass_utils import run_bass_kernel_spmd

    # adaptive tile capacity: expected inputs need TPW=9, but tolerate
    # denser dst windows by bumping the per-window tile count
    dst_a = np.asarray(dst).astype(np.int64)
    wmax = int(np.bincount(dst_a // WSZ, minlength=NCORES * NW).max())
    tpw = max(TPW, -(-wmax // 128))

    in_maps = host_prep(feat, beta, src, dst, NCORES, NW, tpw, D)
    key = ("nc", tpw)
    if key not in _CACHED:
        _CACHED[key] = build_graph(NW, tpw, WB)
    nc = _CACHED[key]
    res = run_bass_kernel_spmd(nc, in_maps, list(range(NCORES))).results
    full = np.concatenate([res[c]["out"] for c in range(NCORES)], axis=0)
    return full[:N_NODES].astype(np.float32)
